# revision 8
# baseline (speedup 1.0000x reference)
"""Bidirectional GRU layer (T=512, B=64, I=H=512) on 8 Trainium2 NeuronCores.

Strategy
--------
The recurrence h_t = GRU(x_t, h_{t-1}) is a 512-step serial chain per
direction. Everything is kept in a transposed layout so that no on-chip
transposes are ever needed:

  * hidden state lives as h.T: SBUF [128 (h mod 128), 4 (h chunk), B_loc]
  * gh.T = Whh @ h.T is computed with the *weights* as the stationary
    operand (lhsT = Whh.T chunks [128,128], bf16 -> fast weight load) and
    h.T chunks as the tiny moving operand (N = B_loc).
  * gi.T = Wih @ x.T is also computed on the PE, batched 8 timesteps at a
    time (moving operand N = 8*B_loc = 128) and prefetched one body ahead;
    these matmuls fill the PE idle time during the elementwise gate tail.
  * gates (sigmoid/tanh/muls/adds) run on ACT + DVE over [128, chunks*B_loc]
    tiles in fp32 (matmul inputs are bf16, PSUM accumulation fp32).

Sharding (8 cores): cores 0-3 run the forward direction on batch quarters,
cores 4-7 run the backward direction (inputs time-flipped on the host) on
batch quarters. All cores run the identical program (SPMD) with different
input tensors.
"""

import numpy as np
import ml_dtypes

T, B, I, H = 512, 64, 512, 512
NCORES = 8
CORES_PER_DIR = 4
B_LOC = B // CORES_PER_DIR  # 16
KC = H // 128               # 4 contraction chunks
MC = 3 * H // 128           # 12 output chunks (0-3 r, 4-7 z, 8-11 n)
TB = 8                      # timesteps per loop body (gi batch)
TPAD = T + TB               # padded time so the final gi prefetch stays in bounds

_CACHE = {}

bf16 = ml_dtypes.bfloat16


# ----------------------------------------------------------------------------
# Kernel build
# ----------------------------------------------------------------------------

def _build_nc(t_steps=T):
    import concourse.bacc as bacc
    import concourse.mybir as mybir
    import concourse.tile as tile
    import concourse.bass as bass

    fp32 = mybir.dt.float32
    bfl = mybir.dt.bfloat16
    Alu = mybir.AluOpType
    Act = mybir.ActivationFunctionType

    tpad = t_steps + TB
    nc = bacc.Bacc()

    # ---- DRAM tensors (per-core) ----
    xT = nc.dram_tensor("xT", [128, KC, tpad, B_LOC], bfl, kind="ExternalInput")
    whhT = nc.dram_tensor("whhT", [128, KC, MC, 128], bfl, kind="ExternalInput")
    wihT = nc.dram_tensor("wihT", [128, KC, MC, 128], bfl, kind="ExternalInput")
    bgi = nc.dram_tensor("bgi", [128, MC], fp32, kind="ExternalInput")
    bhn = nc.dram_tensor("bhn", [128, KC], fp32, kind="ExternalInput")
    h0T = nc.dram_tensor("h0T", [128, KC, B_LOC], fp32, kind="ExternalInput")
    reps = nc.dram_tensor("reps", [1, 1], mybir.dt.uint32, kind="ExternalInput")

    # outputs are stored in the on-chip layout: (kc, b, p) with p contiguous;
    # the host un-permutes. This keeps the per-step DMA a 2-dim AP (required
    # with a dynamic offset) and fully contiguous in DRAM.
    out = nc.dram_tensor("out", [t_steps, KC, B_LOC, 128], fp32,
                         kind="ExternalOutput")
    hT = nc.dram_tensor("hT", [KC, B_LOC, 128], fp32, kind="ExternalOutput")

    hTT = bass.AP(tensor=hT.ap().tensor, offset=0,
                  ap=[[1, 128], [128, KC * B_LOC]])

    def out_step_ap(t_val):
        return bass.AP(
            tensor=out.ap().tensor,
            offset=t_val * (KC * B_LOC * 128),
            ap=[[1, 128], [128, KC * B_LOC]],
        )

    with tile.TileContext(nc) as tc:
        with (
            tc.tile_pool(name="consts", bufs=1) as consts,
            tc.tile_pool(name="state", bufs=1) as state,
            tc.tile_pool(name="tmp", bufs=3) as tmp,
            tc.tile_pool(name="psum", bufs=1, space="PSUM") as psum,
        ):
            # ---- persistent SBUF ----
            whh_sb = consts.tile([128, KC, MC, 128], bfl)
            wih_sb = consts.tile([128, KC, MC, 128], bfl)
            bgi_sb = consts.tile([128, MC], fp32)
            bhn_sb = consts.tile([128, KC], fp32)
            reps_sb = consts.tile([1, 1], mybir.dt.uint32)

            h32 = state.tile([128, 2, KC, B_LOC], fp32)     # fp32 master state
            hbf = state.tile([128, 2, KC, B_LOC], bfl)      # bf16 mirror for PE
            xT_sb = state.tile([128, KC, TB, B_LOC], bfl)   # x.T slice for next body
            gi_sb = state.tile([128, MC, TB * B_LOC], fp32)  # gi for current body

            # ---- persistent PSUM ----
            ps_rz = [psum.tile([128, 8, B_LOC], fp32, name=f"ps_rz{p}") for p in (0, 1)]
            ps_n = [psum.tile([128, 4, B_LOC], fp32, name=f"ps_n{p}") for p in (0, 1)]
            ps_gi = psum.tile([128, MC, TB * B_LOC], fp32, name="ps_gi")

            # ---- one-time loads ----
            nc.sync.dma_start(out=whh_sb[:], in_=whhT.ap())
            nc.sync.dma_start(out=wih_sb[:], in_=wihT.ap())
            nc.sync.dma_start(out=bgi_sb[:], in_=bgi.ap())
            nc.sync.dma_start(out=bhn_sb[:], in_=bhn.ap())
            nc.sync.dma_start(out=reps_sb[:], in_=reps.ap())

            # repeat count -> a register on every engine (for the timing loop)
            regs = []
            for e in mybir.ALL_ENGINES:
                r = nc.alloc_register(e, f"reps_{e.name}")
                nc.engines[e].load(r, reps_sb[0:1, 0:1])
                regs.append(r)
            reps_val = nc.snap(bass.RegisterHandles(regs), min_val=1, max_val=1 << 20)

            bhn_bc = bhn_sb[:, :, None].to_broadcast((128, KC, B_LOC))
            bgi_bc = bgi_sb[:, :, None].to_broadcast((128, MC, TB * B_LOC))

            def gi_batch_mms(t0_val):
                """Emit DMA + 48 matmuls + evac computing gi for steps
                [t0_val, t0_val+TB) into gi_sb. Used for the prologue only."""
                nc.sync.dma_start(
                    out=xT_sb[:], in_=xT.ap()[:, :, bass.ds(t0_val, TB), :]
                )
                for mc in range(MC):
                    for kc in range(KC):
                        nc.tensor.matmul(
                            ps_gi[:, mc, :],
                            wih_sb[:, kc, mc, :],
                            xT_sb[:, kc, :, :],
                            start=(kc == 0),
                            stop=(kc == KC - 1),
                        )
                nc.vector.tensor_tensor(
                    gi_sb[:], ps_gi[:], bgi_bc, Alu.add
                )

            def step(s, t_ap):
                """One GRU timestep. s = position in body (static), t_ap =
                dynamic absolute timestep (for the output DMA). Consumes
                h32/hbf parity s%2, produces parity (s+1)%2."""
                p, q = s % 2, (s + 1) % 2
                prz, pn = ps_rz[p], ps_n[p]
                gs = slice(s * B_LOC, (s + 1) * B_LOC)

                # gh.T for r,z chunks (0..7)
                for mc in range(8):
                    for kc in range(KC):
                        nc.tensor.matmul(
                            prz[:, mc, :],
                            whh_sb[:, kc, mc, :],
                            hbf[:, p, kc, :],
                            start=(kc == 0),
                            stop=(kc == KC - 1),
                        )
                # rz pre-activations + sigmoid
                t_rz = tmp.tile([128, 8, B_LOC], fp32, tag="t_rz")
                nc.vector.tensor_tensor(
                    t_rz[:], prz[:], gi_sb[:, 0:8, gs], Alu.add
                )
                sig = tmp.tile([128, 8, B_LOC], fp32, tag="sig")
                nc.scalar.activation(sig[:], t_rz[:], Act.Sigmoid)
                zm = tmp.tile([128, 4, B_LOC], fp32, tag="zm")
                nc.scalar.activation(zm[:], t_rz[:, 4:8, :], Act.Sigmoid, scale=-1.0)

                # gh.T for n chunks (8..11)
                for mc in range(8, MC):
                    for kc in range(KC):
                        nc.tensor.matmul(
                            pn[:, mc - 8, :],
                            whh_sb[:, kc, mc, :],
                            hbf[:, p, kc, :],
                            start=(kc == 0),
                            stop=(kc == KC - 1),
                        )
                s_hn = tmp.tile([128, 4, B_LOC], fp32, tag="s_hn")
                nc.vector.tensor_tensor(s_hn[:], pn[:], bhn_bc, Alu.add)
                rh = tmp.tile([128, 4, B_LOC], fp32, tag="rh")
                nc.vector.tensor_tensor(rh[:], sig[:, 0:4, :], s_hn[:], Alu.mult)
                npre = tmp.tile([128, 4, B_LOC], fp32, tag="npre")
                nc.vector.tensor_tensor(npre[:], rh[:], gi_sb[:, 8:12, gs], Alu.add)
                n_t = tmp.tile([128, 4, B_LOC], fp32, tag="n_t")
                nc.scalar.activation(n_t[:], npre[:], Act.Tanh)

                # gi matmuls for the next body (6 of the 48 per step)
                for qq in range(6 * s, 6 * s + 6):
                    mc_g, kc_g = qq // KC, qq % KC
                    nc.tensor.matmul(
                        ps_gi[:, mc_g, :],
                        wih_sb[:, kc_g, mc_g, :],
                        xT_sb[:, kc_g, :, :],
                        start=(kc_g == 0),
                        stop=(kc_g == KC - 1),
                    )

                # h' = (1-z)*n + z*h
                u = tmp.tile([128, 4, B_LOC], fp32, tag="u")
                nc.vector.tensor_tensor(u[:], sig[:, 4:8, :], h32[:, p], Alu.mult)
                v = tmp.tile([128, 4, B_LOC], fp32, tag="v")
                nc.vector.tensor_tensor(v[:], n_t[:], zm[:], Alu.mult)
                nc.vector.tensor_tensor(h32[:, q], u[:], v[:], Alu.add)
                nc.scalar.copy(hbf[:, q], h32[:, q])

                nc.sync.dma_start(out=out_step_ap(t_ap), in_=h32[:, q])

            with tc.For_i(0, reps_val, 1, name="reps") as _rep:
                # prologue: load h0, compute gi for body 0
                nc.sync.dma_start(out=h32[:, 0], in_=h0T.ap())
                nc.vector.tensor_copy(hbf[:, 0], h32[:, 0])
                gi_batch_mms(0)

                with tc.For_i(
                    0, t_steps, TB,
                    name="tloop",
                    hint_engines=(mybir.EngineType.PE,),
                ) as iv:
                    # x.T slice for the *next* body (gi prefetch)
                    nc.sync.dma_start(
                        out=xT_sb[:], in_=xT.ap()[:, :, bass.ds(iv + TB, TB), :]
                    )
                    for s in range(TB):
                        step(s, iv + s)
                    # evacuate next body's gi (adds the gi biases)
                    nc.vector.tensor_tensor(gi_sb[:], ps_gi[:], bgi_bc, Alu.add)

                nc.sync.dma_start(out=hTT, in_=h32[:, 0])

    nc.compile()
    return nc


# ----------------------------------------------------------------------------
# Host-side input prep / output assembly
# ----------------------------------------------------------------------------

def _prep_core_inputs(inp, h0_fwd, h0_bwd, Wih_f, Whh_f, bih_f, bhh_f,
                      Wih_b, Whh_b, bih_b, bhh_b, reps=1, t_steps=T):
    """Build the 8 per-core input dicts."""
    def wT(W):
        # [p, kc, mc, m] = W[mc*128+m, kc*128+p]
        return np.ascontiguousarray(
            W.reshape(MC, 128, KC, 128).transpose(3, 2, 0, 1)
        ).astype(bf16)

    def bias_gi(bih, bhh):
        v = np.concatenate([bih[: 2 * H] + bhh[: 2 * H], bih[2 * H:]])
        return np.ascontiguousarray(v.reshape(MC, 128).T).astype(np.float32)

    def bias_hn(bhh):
        return np.ascontiguousarray(bhh[2 * H:].reshape(KC, 128).T).astype(np.float32)

    def xT_of(x):
        # x [t_steps, B, I] -> [p, kc, tpad, B] bf16 (zero padded in t)
        xt = np.zeros((128, KC, t_steps + TB, B), dtype=bf16)
        xt[:, :, :t_steps, :] = (
            x.reshape(t_steps, B, KC, 128).transpose(3, 2, 0, 1).astype(bf16)
        )
        return xt

    def h0T_of(h0):
        # h0 [B, H] -> [p, kc, B] fp32
        return np.ascontiguousarray(
            h0.reshape(B, KC, 128).transpose(2, 1, 0)
        ).astype(np.float32)

    reps_arr = np.array([[reps]], dtype=np.uint32)

    in_maps = []
    for d, (x_d, h0_d, wih, whh, bi, bh) in enumerate([
        (inp, h0_fwd, Wih_f, Whh_f, (bih_f, bhh_f), None),
        (inp[::-1], h0_bwd, Wih_b, Whh_b, (bih_b, bhh_b), None),
    ]):
        xt_full = xT_of(np.ascontiguousarray(x_d))
        h0t_full = h0T_of(h0_d)
        wihT_d, whhT_d = wT(wih), wT(whh)
        bgi_d = bias_gi(bi[0], bi[1])
        bhn_d = bias_hn(bi[1])
        for c in range(CORES_PER_DIR):
            bs = slice(c * B_LOC, (c + 1) * B_LOC)
            in_maps.append({
                "xT": np.ascontiguousarray(xt_full[:, :, :, bs]),
                "whhT": whhT_d,
                "wihT": wihT_d,
                "bgi": bgi_d,
                "bhn": bhn_d,
                "h0T": np.ascontiguousarray(h0t_full[:, :, bs]),
                "reps": reps_arr,
            })
    return in_maps


def _assemble(results, t_steps=T):
    out = np.empty((t_steps, B, 2 * H), dtype=np.float32)
    hT_f = np.empty((B, H), dtype=np.float32)
    hT_b = np.empty((B, H), dtype=np.float32)
    def un_t(o):
        # [t, KC, B_LOC, 128] -> [t, B_LOC, H]
        return o.transpose(0, 2, 1, 3).reshape(o.shape[0], B_LOC, H)

    def un_h(o):
        # [KC, B_LOC, 128] -> [B_LOC, H]
        return o.transpose(1, 0, 2).reshape(B_LOC, H)

    for c in range(CORES_PER_DIR):
        bs = slice(c * B_LOC, (c + 1) * B_LOC)
        out[:, bs, :H] = un_t(results[c]["out"])
        out[:, bs, H:] = un_t(results[CORES_PER_DIR + c]["out"])[::-1]
        hT_f[bs] = un_h(results[c]["hT"])
        hT_b[bs] = un_h(results[CORES_PER_DIR + c]["hT"])
    return out, hT_f, hT_b


def _get_nc(t_steps=T):
    key = ("nc", t_steps)
    if key not in _CACHE:
        _CACHE[key] = _build_nc(t_steps)
    return _CACHE[key]


def _run(in_maps, t_steps=T):
    from concourse.bass_utils import run_bass_kernel_spmd
    nc = _get_nc(t_steps)
    return run_bass_kernel_spmd(nc, in_maps, core_ids=list(range(NCORES)))


def kernel(**inputs):
    in_maps = _prep_core_inputs(**{k: np.asarray(v) for k, v in inputs.items()})
    res = _run(in_maps)
    return _assemble(res.results)


# revision 15
# speedup vs baseline: 9.9708x; 9.9708x over previous
"""Bidirectional GRU layer (T=512, B=64, I=H=512) on 8 Trainium2 NeuronCores.

Strategy
--------
The recurrence h_t = GRU(x_t, h_{t-1}) is a 512-step serial chain per
direction. Everything is kept in a transposed layout so that no on-chip
transposes are ever needed:

  * hidden state lives as h.T: SBUF [128 (h mod 128), 4 (h chunk), b]
  * gh.T = Whh @ h.T is computed with the *weights* as the stationary
    operand (lhsT = Whh.T chunks [128,128], bf16 -> fast weight load) and
    h.T chunks as the tiny moving operand.
  * gi.T = Wih @ x.T is also computed on the PE, batched 8 timesteps at a
    time and prefetched one block ahead; these matmuls fill PE idle time.
  * gates (sigmoid/tanh/muls/adds) run on ACT + DVE in fp32.
  * the batch is split into NSUB interleaved substreams per core: while the
    PE runs substream A's matmuls, substream B's gate tail runs on DVE/ACT,
    keeping the PE near 100% busy.
  * outputs are staged in SBUF and written with ONE DMA per 8 steps
    (per-step dynamic DMAs measured ~40us each on this runtime).

Sharding (8 cores): cores 0-3 run the forward direction on batch quarters,
cores 4-7 run the backward direction (inputs time-flipped on the host) on
batch quarters. All cores run the identical program (SPMD).
"""

import numpy as np
import ml_dtypes

T, B, I, H = 512, 64, 512, 512
NCORES = 8
CORES_PER_DIR = 4
B_LOC = B // CORES_PER_DIR  # 16
KC = H // 128               # 4 contraction chunks
MC = 3 * H // 128           # 12 output chunks (0-3 r, 4-7 z, 8-11 n)
TB = 8                      # timesteps per gi batch / output block
TPAD = T + 2 * TB           # padded time so gi prefetch stays in bounds
NSUB = 2                    # interleaved substreams per core

_CACHE = {}

bf16 = ml_dtypes.bfloat16


# ----------------------------------------------------------------------------
# Kernel build
# ----------------------------------------------------------------------------

def _build_nc(t_steps=T, ablate=(), nsub=NSUB):
    """ablate: feature names to disable (timing experiments only):
    'out_dma' | 'gi' | 'tail' | 'gh'
    """
    import concourse.bacc as bacc
    import concourse.mybir as mybir
    import concourse.tile as tile
    import concourse.bass as bass

    fp32 = mybir.dt.float32
    bfl = mybir.dt.bfloat16
    Alu = mybir.AluOpType
    Act = mybir.ActivationFunctionType

    BS = B_LOC // nsub
    tpad = t_steps + 2 * TB
    assert t_steps % (2 * TB) == 0
    use_gi = "gi" not in ablate
    use_tail = "tail" not in ablate
    use_gh = "gh" not in ablate
    use_out = "out_dma" not in ablate

    nc = bacc.Bacc()

    # ---- DRAM tensors (per-core) ----
    xT = nc.dram_tensor("xT", [128, KC, tpad, B_LOC], bfl, kind="ExternalInput")
    whhT = nc.dram_tensor("whhT", [128, KC, MC, 128], bfl, kind="ExternalInput")
    wihT = nc.dram_tensor("wihT", [128, KC, MC, 128], bfl, kind="ExternalInput")
    bgi = nc.dram_tensor("bgi", [128, MC], fp32, kind="ExternalInput")
    bhn = nc.dram_tensor("bhn", [128, KC], fp32, kind="ExternalInput")
    h0T = nc.dram_tensor("h0T", [128, KC, B_LOC], fp32, kind="ExternalInput")
    reps = nc.dram_tensor("reps", [1, 1], mybir.dt.uint32, kind="ExternalInput")

    # outputs in on-chip layout, host un-permutes:
    #   out[sub, t, kc, b, p] = h_t[b_global = sub*BS+b, kc*128+p]
    out = nc.dram_tensor("out", [nsub, t_steps, KC, BS, 128], fp32,
                         kind="ExternalOutput")
    hT = nc.dram_tensor("hT", [nsub, KC, BS, 128], fp32, kind="ExternalOutput")

    BLK = TB * KC * BS * 128  # elements per (sub, 8-step block) of `out`

    def out_blk_ap(sub, t0_val):
        return bass.AP(
            tensor=out.ap().tensor,
            offset=sub * (t_steps * KC * BS * 128) + t0_val * (KC * BS * 128),
            ap=[[1, 128], [128, TB * KC * BS]],
        )

    def hT_ap(sub):
        return bass.AP(
            tensor=hT.ap().tensor,
            offset=sub * (KC * BS * 128),
            ap=[[1, 128], [128, KC * BS]],
        )

    with tile.TileContext(nc) as tc:
        with (
            tc.tile_pool(name="consts", bufs=1) as consts,
            tc.tile_pool(name="state", bufs=1) as state,
            tc.tile_pool(name="tmp", bufs=3) as tmp,
            tc.tile_pool(name="psum", bufs=1, space="PSUM") as psum,
        ):
            # ---- persistent SBUF ----
            whh_sb = consts.tile([128, KC, MC, 128], bfl)
            wih_sb = consts.tile([128, KC, MC, 128], bfl)
            bgi_sb = consts.tile([128, MC], fp32)
            bhn_sb = consts.tile([128, KC], fp32)
            reps_sb = consts.tile([1, 1], mybir.dt.uint32)

            # bf16 state mirrors (PE operand), per substream, step parity
            hbf = [state.tile([128, 2, KC, BS], bfl, name=f"hbf{u}")
                   for u in range(nsub)]
            # fp32 state ring = output staging: [TB slots, KC, BS] x2 blocks
            stg = [[state.tile([128, TB, KC, BS], fp32, name=f"stg{u}_{j}")
                    for j in range(2)] for u in range(nsub)]
            if use_gi:
                xT_sb = [state.tile([128, KC, TB, B_LOC], bfl, name=f"xT{j}")
                         for j in range(2)]
                ps_gi = psum.tile([128, MC, TB * B_LOC], fp32, name="ps_gi")
            if use_gi or use_tail:
                gi_sb = [state.tile([128, MC, TB * B_LOC], fp32, name=f"gi{j}")
                         for j in range(2)]

            # ---- persistent PSUM: one tile (bank) per (substream, parity),
            # holding all 12 output chunks ----
            if use_gh or use_tail:
                ps = [[psum.tile([128, MC, BS], fp32, name=f"ps{u}_{p}")
                       for p in (0, 1)] for u in range(nsub)]

            # ---- one-time loads ----
            nc.sync.dma_start(out=whh_sb[:], in_=whhT.ap())
            nc.sync.dma_start(out=wih_sb[:], in_=wihT.ap())
            nc.sync.dma_start(out=bgi_sb[:], in_=bgi.ap())
            nc.sync.dma_start(out=bhn_sb[:], in_=bhn.ap())
            nc.sync.dma_start(out=reps_sb[:], in_=reps.ap())

            # repeat count -> a register on every engine (for the timing loop)
            regs = []
            for e in mybir.ALL_ENGINES:
                r = nc.alloc_register(e, f"reps_{e.name}")
                nc.engines[e].load(r, reps_sb[0:1, 0:1])
                regs.append(r)
            reps_val = nc.snap(bass.RegisterHandles(regs), min_val=1,
                               max_val=1 << 20)

            bhn_bc = bhn_sb[:, :, None].to_broadcast((128, KC, BS))
            bgi_bc = bgi_sb[:, :, None].to_broadcast((128, MC, TB * B_LOC))

            def gi_mms(jdst, mm_range):
                """gi matmuls (subset mm_range of the 48) for block jdst."""
                for qq in mm_range:
                    mc_g, kc_g = qq // KC, qq % KC
                    nc.tensor.matmul(
                        ps_gi[:, mc_g, :],
                        wih_sb[:, kc_g, mc_g, :],
                        xT_sb[jdst][:, kc_g, :, :],
                        start=(kc_g == 0),
                        stop=(kc_g == KC - 1),
                    )

            def gi_evac(jdst):
                nc.vector.tensor_tensor(gi_sb[jdst][:], ps_gi[:], bgi_bc,
                                        Alu.add)

            def step(sub, j, s):
                """One GRU timestep for substream `sub`, block `j`, slot `s`.
                Reads h from stg[sub][j][s-1] (or stg[sub][j^1][TB-1]),
                writes stg[sub][j][s] and hbf parity (s+1)%2."""
                p, q = s % 2, (s + 1) % 2
                pst = ps[sub][p]
                gs = slice(s * B_LOC + sub * BS, s * B_LOC + (sub + 1) * BS)
                h_prev = (stg[sub][j][:, s - 1] if s > 0
                          else stg[sub][j ^ 1][:, TB - 1])

                if use_gh:
                    for mc in range(MC):
                        for kc in range(KC):
                            nc.tensor.matmul(
                                pst[:, mc, :],
                                whh_sb[:, kc, mc, :],
                                hbf[sub][:, p, kc, :],
                                start=(kc == 0),
                                stop=(kc == KC - 1),
                            )
                if not use_tail:
                    return
                t_rz = tmp.tile([128, 8, BS], fp32, tag="t_rz")
                nc.vector.tensor_tensor(t_rz[:], pst[:, 0:8, :],
                                        gi_sb[j][:, 0:8, gs], Alu.add)
                sig = tmp.tile([128, 8, BS], fp32, tag="sig")
                nc.scalar.activation(sig[:], t_rz[:], Act.Sigmoid)
                zm = tmp.tile([128, 4, BS], fp32, tag="zm")
                nc.scalar.activation(zm[:], t_rz[:, 4:8, :], Act.Sigmoid,
                                     scale=-1.0)
                s_hn = tmp.tile([128, 4, BS], fp32, tag="s_hn")
                nc.vector.tensor_tensor(s_hn[:], pst[:, 8:12, :], bhn_bc,
                                        Alu.add)
                rh = tmp.tile([128, 4, BS], fp32, tag="rh")
                nc.vector.tensor_tensor(rh[:], sig[:, 0:4, :], s_hn[:],
                                        Alu.mult)
                npre = tmp.tile([128, 4, BS], fp32, tag="npre")
                nc.vector.tensor_tensor(npre[:], rh[:],
                                        gi_sb[j][:, 8:12, gs], Alu.add)
                n_t = tmp.tile([128, 4, BS], fp32, tag="n_t")
                nc.scalar.activation(n_t[:], npre[:], Act.Tanh)

                u_t = tmp.tile([128, 4, BS], fp32, tag="u_t")
                nc.vector.tensor_tensor(u_t[:], sig[:, 4:8, :], h_prev,
                                        Alu.mult)
                v_t = tmp.tile([128, 4, BS], fp32, tag="v_t")
                nc.vector.tensor_tensor(v_t[:], n_t[:], zm[:], Alu.mult)
                # bf16 mirror first (unblocks the next step's matmuls)
                nc.vector.tensor_tensor(hbf[sub][:, q], u_t[:], v_t[:],
                                        Alu.add)
                nc.vector.tensor_tensor(stg[sub][j][:, s], u_t[:], v_t[:],
                                        Alu.add)

            with tc.For_i(0, reps_val, 1, name="reps") as _rep:
                # prologue: h0 -> stg[*][1][TB-1] + hbf parity 0;
                # gi for block 0
                for sub in range(nsub):
                    bsl = slice(sub * BS, (sub + 1) * BS)
                    nc.sync.dma_start(out=stg[sub][1][:, TB - 1],
                                      in_=h0T.ap()[:, :, bsl])
                    nc.vector.tensor_copy(hbf[sub][:, 0],
                                          stg[sub][1][:, TB - 1])
                if use_gi:
                    nc.sync.dma_start(out=xT_sb[0][:],
                                      in_=xT.ap()[:, :, 0:TB, :])
                    gi_mms(0, range(48))
                    gi_evac(0)
                elif use_tail:
                    for j in range(2):
                        nc.vector.memset(gi_sb[j][:], 0.0)

                with tc.For_i(
                    0, t_steps, 2 * TB,
                    name="tloop",
                    hint_engines=(mybir.EngineType.PE,),
                ) as iv:
                    for j in range(2):
                        # prefetch x.T for the next block into xT_sb[j^1]
                        if use_gi:
                            nc.sync.dma_start(
                                out=xT_sb[j ^ 1][:],
                                in_=xT.ap()[:, :,
                                            bass.ds(iv + TB * (j + 1), TB), :],
                            )
                        for s in range(TB):
                            for sub in range(nsub):
                                step(sub, j, s)
                            # spread next block's gi matmuls across the steps
                            if use_gi:
                                gi_mms(j ^ 1, range(6 * s, 6 * s + 6))
                        if use_gi:
                            gi_evac(j ^ 1)
                        if use_out:
                            for sub in range(nsub):
                                nc.sync.dma_start(
                                    out=out_blk_ap(sub, iv + TB * j),
                                    in_=stg[sub][j][:],
                                )

                for sub in range(nsub):
                    nc.sync.dma_start(out=hT_ap(sub),
                                      in_=stg[sub][1][:, TB - 1])

    nc.compile()
    return nc


# ----------------------------------------------------------------------------
# Host-side input prep / output assembly
# ----------------------------------------------------------------------------

def _prep_core_inputs(inp, h0_fwd, h0_bwd, Wih_f, Whh_f, bih_f, bhh_f,
                      Wih_b, Whh_b, bih_b, bhh_b, reps=1, t_steps=T):
    """Build the 8 per-core input dicts."""
    def wT(W):
        # [p, kc, mc, m] = W[mc*128+m, kc*128+p]
        return np.ascontiguousarray(
            W.reshape(MC, 128, KC, 128).transpose(3, 2, 0, 1)
        ).astype(bf16)

    def bias_gi(bih, bhh):
        v = np.concatenate([bih[: 2 * H] + bhh[: 2 * H], bih[2 * H:]])
        return np.ascontiguousarray(v.reshape(MC, 128).T).astype(np.float32)

    def bias_hn(bhh):
        return np.ascontiguousarray(
            bhh[2 * H:].reshape(KC, 128).T).astype(np.float32)

    def xT_of(x):
        # x [t_steps, B, I] -> [p, kc, tpad, B] bf16 (zero padded in t)
        xt = np.zeros((128, KC, t_steps + 2 * TB, B), dtype=bf16)
        xt[:, :, :t_steps, :] = (
            x.reshape(t_steps, B, KC, 128).transpose(3, 2, 0, 1).astype(bf16)
        )
        return xt

    def h0T_of(h0):
        # h0 [B, H] -> [p, kc, B] fp32
        return np.ascontiguousarray(
            h0.reshape(B, KC, 128).transpose(2, 1, 0)
        ).astype(np.float32)

    reps_arr = np.array([[reps]], dtype=np.uint32)

    in_maps = []
    for x_d, h0_d, wih, whh, bih, bhh in [
        (inp, h0_fwd, Wih_f, Whh_f, bih_f, bhh_f),
        (inp[::-1], h0_bwd, Wih_b, Whh_b, bih_b, bhh_b),
    ]:
        xt_full = xT_of(np.ascontiguousarray(x_d))
        h0t_full = h0T_of(h0_d)
        wihT_d, whhT_d = wT(wih), wT(whh)
        bgi_d = bias_gi(bih, bhh)
        bhn_d = bias_hn(bhh)
        for c in range(CORES_PER_DIR):
            bs = slice(c * B_LOC, (c + 1) * B_LOC)
            in_maps.append({
                "xT": np.ascontiguousarray(xt_full[:, :, :, bs]),
                "whhT": whhT_d,
                "wihT": wihT_d,
                "bgi": bgi_d,
                "bhn": bhn_d,
                "h0T": np.ascontiguousarray(h0t_full[:, :, bs]),
                "reps": reps_arr,
            })
    return in_maps


def _assemble(results, t_steps=T):
    out = np.empty((t_steps, B, 2 * H), dtype=np.float32)
    hT_f = np.empty((B, H), dtype=np.float32)
    hT_b = np.empty((B, H), dtype=np.float32)

    def un_t(o):
        # [nsub, t, KC, BS, 128] -> [t, B_LOC, H]
        ns, tt = o.shape[0], o.shape[1]
        return o.transpose(1, 0, 3, 2, 4).reshape(tt, B_LOC, H)

    def un_h(o):
        # [nsub, KC, BS, 128] -> [B_LOC, H]
        return o.transpose(0, 2, 1, 3).reshape(B_LOC, H)

    for c in range(CORES_PER_DIR):
        bs = slice(c * B_LOC, (c + 1) * B_LOC)
        out[:, bs, :H] = un_t(results[c]["out"])
        out[:, bs, H:] = un_t(results[CORES_PER_DIR + c]["out"])[::-1]
        hT_f[bs] = un_h(results[c]["hT"])
        hT_b[bs] = un_h(results[CORES_PER_DIR + c]["hT"])
    return out, hT_f, hT_b


def _get_nc(t_steps=T, ablate=(), nsub=NSUB):
    key = ("nc", t_steps, tuple(sorted(ablate)), nsub)
    if key not in _CACHE:
        _CACHE[key] = _build_nc(t_steps, ablate, nsub)
    return _CACHE[key]


def _run(in_maps, t_steps=T, ablate=(), nsub=NSUB):
    from concourse.bass_utils import run_bass_kernel_spmd
    nc = _get_nc(t_steps, ablate, nsub)
    return run_bass_kernel_spmd(nc, in_maps, core_ids=list(range(NCORES)))


def kernel(**inputs):
    in_maps = _prep_core_inputs(**{k: np.asarray(v) for k, v in inputs.items()})
    res = _run(in_maps)
    return _assemble(res.results)
